# revision 11
# baseline (speedup 1.0000x reference)
"""Trainium2 Bass kernel for nn_DendriticBranchLayer.

rate = alpha * relu(V - Vth)^2,  V = (exc + cur) / (exc + 1 + cond + inh)
  exc = x @ pruned(pre_w_exc, K=32).T        [B, OUT]
  inh = inhibitory_input @ pruned(pre_w_inh, K=16).T
  cur = sum_f branch_input.reshape(B,OUT,4)[...,f] * w_block[:,f]

Strategy: batch sharded over 8 cores. The masked weights depend only on the
small weight tensors, so dense masked weights are materialized on the host.
Both matmuls run in fp8 E4M3 with perf_mode=DoubleRow (two contraction rows
per PE cell -> 2x matmul throughput vs fp16; measured 512-col DoubleRow MMs
stream at ~216ns = 1 col/cycle with 256-deep contraction). The weights all
lie in exp([-2.1, -2.0]) — a 10%-wide band — so they are rescaled by s to sit
in [0.895, 0.99] where the E4M3 grid is finest, and x is pre-divided by s on
the host (exact scale cancellation). Emulated end-to-end rel_l2: ~1.2e-2
(gate 2e-2). branch_input stays fp16 (it dominates the numerator).

Outputs live on PSUM partitions (128 outputs/block) with 512 batch on the
free dim: one PSUM bank per accumulator, 4+4 banks of double buffering.
Pointwise per (nb, ob) is spread across engines so it hides under the tensor
engine (~83us of matmul stream):
  GpSimd: t0 = br0+br1 ; t1 = br2+br3          (fp16, SBUF-only engine)
  DVE:    cur = t0+t1 ; num = exc_ps + cur ; den = (exc_ps + c) + inh1 ;
          rden = 1/den ; v = num * rden
  ACT:    inh1 = copy(inh_ps) ; r = relu(v - Vth) ; out = (sa*r)^2 -> fp16

DMA is issued from both HWDGE rings (sync + scalar engines) so descriptor
generation is not serialized on one queue; weights are shipped as one merged
768KB tensor per output block. The last iteration runs as two half-batch
pieces so its pointwise tail is half as long.
"""

import numpy as np

import concourse.bass as bass
import concourse.mybir as mybir
import concourse.tile as tile
from concourse import bacc
from concourse.bass_utils import run_bass_kernel_spmd

B, OUT, EXC_IN, INH_IN, BF = 8192, 1024, 4096, 2048, 4
K_EXC, K_INH = 32, 16

NCORES = 8
BC = B // NCORES          # batch per core (1024)
P = 128                   # partitions
NB = 2                    # batch sub-blocks per core
BSUB = BC // NB           # 512 batch per sub-block (one PSUM bank)
OB = OUT // P             # 8 output blocks
KE = EXC_IN // P          # 32 contraction chunks (exc)
KI = INH_IN // P          # 16 contraction chunks (inh)
KW = KE + KI              # merged weight chunks per output block
XCH = 4                   # DMA chunks per x sub-block (8 k-chunks each)
ICH = 4                   # DMA chunks per ih sub-block (4 k-chunks each)

# weight rescale: exp([-2.1,-2.0]) * WS lands in [0.895, 0.989] where the
# E4M3 grid step is 1/16
WS = float(0.9375 / np.exp(-2.05))

# cst column layout: [P, 3*OB]
_C_CP1 = 0                # 1 + cond, per output
_C_VTHN = OB              # -Vth, per output
_C_SA = 2 * OB            # sqrt(alpha), per output
_C_COLS = 3 * OB

_CACHE = {}
TRACE = False  # set by test harness to capture an NTFF profile


def _build_program():
    nc = bacc.Bacc("TRN2", target_bir_lowering=False, debug=False)
    f8, f16, f32 = mybir.dt.float8e4, mybir.dt.float16, mybir.dt.float32
    DR = mybir.MatmulPerfMode.DoubleRow

    wt = nc.declare_dram_parameter("wt", [P, OB, KW, P], f8, isOutput=False)
    xt = nc.declare_dram_parameter("xt", [NB, XCH, P, KE // XCH, BSUB], f8,
                                   isOutput=False)
    iht = nc.declare_dram_parameter("iht", [NB, ICH, P, KI // ICH, BSUB], f8,
                                    isOutput=False)
    brt = nc.declare_dram_parameter("brt", [NB, OB, P, BF, BSUB], f16,
                                    isOutput=False)
    cst = nc.declare_dram_parameter("cst", [P, _C_COLS], f32, isOutput=False)
    outt = nc.declare_dram_parameter("outt", [OB, P, NB, BSUB], f16,
                                     isOutput=True)

    add = mybir.AluOpType.add
    Relu = mybir.ActivationFunctionType.Relu
    Square = mybir.ActivationFunctionType.Square
    Identity = mybir.ActivationFunctionType.Identity

    KEC = KE // XCH           # k-chunks per x DMA chunk (8)
    KIC = KI // ICH           # k-chunks per ih DMA chunk (4)

    with tile.TileContext(nc) as tc:
        with tc.tile_pool(name="wpool", bufs=1) as wpool, \
             tc.tile_pool(name="xpool", bufs=1) as xpool, \
             tc.tile_pool(name="brpool", bufs=5) as brpool, \
             tc.tile_pool(name="wk", bufs=2) as wk, \
             tc.tile_pool(name="opool", bufs=3) as opool, \
             tc.tile_pool(name="ps_exc", bufs=3, space="PSUM") as ps_exc, \
             tc.tile_pool(name="ps_inh", bufs=3, space="PSUM") as ps_inh:

            cst_s = wpool.tile([P, _C_COLS], f32)
            wt_sb = [None] * OB
            xt_sb = [[None] * XCH for _ in range(NB)]
            iht_sb = [[None] * ICH for _ in range(NB)]

            def load_w(ob):
                if ob >= OB or wt_sb[ob] is not None:
                    return
                w = wpool.tile([P, KW, P], f8, tag=f"wt{ob}")
                nc.sync.dma_start(out=w, in_=wt[:, ob, :, :])
                wt_sb[ob] = w

            def load_x(nb, c, eng):
                if xt_sb[nb][c] is not None:
                    return
                xs = xpool.tile([P, KEC, BSUB], f8, tag=f"xt{nb}c{c}")
                eng.dma_start(out=xs, in_=xt[nb, c, :, :, :])
                xt_sb[nb][c] = xs

            def load_ih(nb, c, eng):
                if iht_sb[nb][c] is not None:
                    return
                ihs = xpool.tile([P, KIC, BSUB], f8, tag=f"iht{nb}c{c}")
                eng.dma_start(out=ihs, in_=iht[nb, c, :, :, :])
                iht_sb[nb][c] = ihs

            # critical lead-in, alternating between the two HWDGE rings.
            # ob0 weights as two tiles so the first inh matmuls only gate on
            # the small wti0 slice; sync ring: weights + x, scalar ring:
            # ih + cst (+ br later)
            wi0 = wpool.tile([P, KI, P], f8, tag="wti0")
            nc.sync.dma_start(out=wi0, in_=wt[:, 0, KE:KW, :])
            load_ih(0, 0, nc.sync)
            load_x(0, 0, nc.sync)
            we0 = wpool.tile([P, KE, P], f8, tag="wte0")
            nc.sync.dma_start(out=we0, in_=wt[:, 0, 0:KE, :])
            wt_sb[0] = (we0, wi0)
            for c in range(1, XCH):
                load_x(0, c, nc.sync)
            for c in range(1, ICH):
                load_ih(0, c, nc.scalar)
            nc.scalar.dma_start(out=cst_s, in_=cst[:, :])

            def emit_exc(exc_ps, nb, ob, cs):
                w = wt_sb[ob]
                we = w[0] if isinstance(w, tuple) else w
                for j in range(KE // 2):            # 16 DoubleRow matmuls
                    c, jj = divmod(j, KEC // 2)
                    nc.tensor.matmul(
                        exc_ps,
                        we[:, 2 * j:2 * j + 2, :],
                        xt_sb[nb][c][:, 2 * jj:2 * jj + 2, cs],
                        start=(j == 0), stop=(j == KE // 2 - 1),
                        perf_mode=DR)

            def emit_inh(inh_ps, nb, ob, cs):
                w = wt_sb[ob]
                wi, k0 = (w[1], 0) if isinstance(w, tuple) else (w, KE)
                for j in range(KI // 2):            # 8 DoubleRow matmuls
                    c, jj = divmod(j, KIC // 2)
                    nc.tensor.matmul(
                        inh_ps,
                        wi[:, k0 + 2 * j:k0 + 2 * j + 2, :],
                        iht_sb[nb][c][:, 2 * jj:2 * jj + 2, cs],
                        start=(j == 0), stop=(j == KI // 2 - 1),
                        perf_mode=DR)

            def pointwise(nb, ob, cs, w, sfx, br_s, exc_ps, inh_ps):
                # cur = sum of the 4 branch inputs (w_block folded on host)
                t0 = wk.tile([P, w], f16, tag="t0" + sfx)
                nc.gpsimd.tensor_add(t0, br_s[:, 0, cs], br_s[:, 1, cs])
                t1 = wk.tile([P, w], f16, tag="t1" + sfx)
                nc.gpsimd.tensor_add(t1, br_s[:, 2, cs], br_s[:, 3, cs])
                cur = wk.tile([P, w], f16, tag="cur" + sfx)
                nc.gpsimd.tensor_add(cur, t0, t1)

                num = wk.tile([P, w], f32, tag="num" + sfx)
                nc.vector.tensor_add(num, exc_ps, cur)
                inh1 = wk.tile([P, w], f32, tag="inh1" + sfx)
                nc.scalar.activation(inh1, inh_ps, Identity)
                den = wk.tile([P, w], f32, tag="den" + sfx)
                nc.vector.scalar_tensor_tensor(
                    den, exc_ps, cst_s[:, _C_CP1 + ob:_C_CP1 + ob + 1], inh1,
                    op0=add, op1=add)
                rden = wk.tile([P, w], f32, tag="rden" + sfx)
                nc.vector.reciprocal_approx_fast(rden, den)
                v = wk.tile([P, w], f32, tag="v" + sfx)
                nc.vector.tensor_mul(v, num, rden)
                r = wk.tile([P, w], f32, tag="r" + sfx)
                nc.scalar.activation(
                    r, v, Relu, bias=cst_s[:, _C_VTHN + ob:_C_VTHN + ob + 1])
                ot = opool.tile([P, w], f16, tag="ot" + sfx)
                nc.scalar.activation(
                    ot, r, Square, scale=cst_s[:, _C_SA + ob:_C_SA + ob + 1])
                nc.sync.dma_start(out=outt[ob, :, nb, cs], in_=ot)

            iters = [(n, o) for n in range(NB) for o in range(OB)]
            for it, (nb, ob) in enumerate(iters):
                last = it == len(iters) - 1
                br_s = brpool.tile([P, BF, BSUB], f16, tag="br")
                nc.scalar.dma_start(out=br_s, in_=brt[nb, ob, :, :, :])

                # prefetch: weights early (scalar ring), second batch half
                # by mid-sweep
                if it == 0:
                    load_w(1)
                elif it == 1:
                    load_w(2), load_w(3)
                elif it == 2:
                    for o in range(4, OB):
                        load_w(o)
                elif it in (3, 4, 5, 6):
                    load_x(1, it - 3, nc.sync)
                    load_ih(1, it - 3, nc.sync)

                exc_ps = ps_exc.tile([P, BSUB], f32, tag="exc")
                inh_ps = ps_inh.tile([P, BSUB], f32, tag="inh")
                full = slice(0, BSUB)
                # inh first: the pointwise chain gates on the exc group, so
                # exc must be the last-emitted matmul group of the iteration
                emit_inh(inh_ps, nb, ob, full)
                emit_exc(exc_ps, nb, ob, full)
                if last:
                    # split the final chain so the kernel tail is shorter
                    h = BSUB // 2
                    pointwise(nb, ob, slice(0, h), h, "a", br_s,
                              exc_ps[:, 0:h], inh_ps[:, 0:h])
                    pointwise(nb, ob, slice(h, BSUB), h, "b", br_s,
                              exc_ps[:, h:BSUB], inh_ps[:, h:BSUB])
                else:
                    pointwise(nb, ob, full, BSUB, "", br_s, exc_ps, inh_ps)

    nc.compile()
    return nc


def _pruned_dense_T(pre_w, K):
    """Masked weight, transposed to [in, out] fp32. Tie-break matches
    jax.lax.top_k: equal values -> lower index wins (stable sort)."""
    idx = np.argsort(-pre_w, axis=1, kind="stable")[:, :K]
    w = np.exp(pre_w.astype(np.float32))
    dense = np.zeros(pre_w.shape, dtype=np.float32)
    np.put_along_axis(dense, idx, np.take_along_axis(w, idx, axis=1), axis=1)
    return dense.T


def kernel(x, inhibitory_input, branch_input, pre_w_exc, pre_w_inh,
           w_block, presigmoid_Vth, log_alpha_max):
    if "nc" not in _CACHE:
        _CACHE["nc"] = _build_program()
    nc = _CACHE["nc"]
    f8np = mybir.dt.np(mybir.dt.float8e4)

    x = np.ascontiguousarray(np.asarray(x, dtype=np.float32))
    inh = np.ascontiguousarray(np.asarray(inhibitory_input, dtype=np.float32))
    br = np.ascontiguousarray(np.asarray(branch_input, dtype=np.float32))
    pre_w_exc = np.asarray(pre_w_exc, dtype=np.float32)
    pre_w_inh = np.asarray(pre_w_inh, dtype=np.float32)
    w_block = np.asarray(w_block, dtype=np.float32)
    presigmoid_Vth = np.asarray(presigmoid_Vth, dtype=np.float32)
    log_alpha_max = np.asarray(log_alpha_max, dtype=np.float32)

    # --- replicated operands -------------------------------------------------
    # wt[p, ob, k, o] = WS * W[ob*P + o, k*P + p], fp8; exc chunks 0..KE-1,
    # inh chunks KE..KW-1
    we_t = (_pruned_dense_T(pre_w_exc, K_EXC) * WS).astype(f8np)
    wi_t = (_pruned_dense_T(pre_w_inh, K_INH) * WS).astype(f8np)
    wte = we_t.reshape(KE, P, OB, P).transpose(1, 2, 0, 3)
    wti = wi_t.reshape(KI, P, OB, P).transpose(1, 2, 0, 3)
    wt = np.ascontiguousarray(np.concatenate([wte, wti], axis=2))

    cond = w_block.sum(axis=1, dtype=np.float32)              # [OUT]
    vth = (1.0 / (1.0 + np.exp(-presigmoid_Vth.astype(np.float64)))).astype(np.float32)
    sa = np.sqrt(np.exp(log_alpha_max.astype(np.float32)))
    cst = np.zeros((P, _C_COLS), dtype=np.float32)
    cst[:, _C_CP1:_C_CP1 + OB] = (1.0 + cond).reshape(OB, P).T
    cst[:, _C_VTHN:_C_VTHN + OB] = (-vth).reshape(OB, P).T
    cst[:, _C_SA:_C_SA + OB] = sa.reshape(OB, P).T

    # fold w_block into branch_input (it is all-ones in practice: skip)
    if not np.all(w_block == 1.0):
        br = (br.reshape(B, OUT, BF) * w_block[None]).reshape(B, OUT * BF)

    inv_s = np.float32(1.0 / WS)
    # --- per-core shards -----------------------------------------------------
    in_maps = []
    for c in range(NCORES):
        s = slice(c * BC, (c + 1) * BC)
        # xt[nb, ch, p, k, b] = x[c*BC + nb*BSUB + b, ((ch*KEC)+k)*P + p] / WS
        xtc = np.ascontiguousarray(
            (x[s] * inv_s).astype(f8np)
            .reshape(NB, BSUB, XCH, KE // XCH, P).transpose(0, 2, 4, 3, 1))
        ihtc = np.ascontiguousarray(
            (inh[s] * inv_s).astype(f8np)
            .reshape(NB, BSUB, ICH, KI // ICH, P).transpose(0, 2, 4, 3, 1))
        # brt[nb, ob, p, f, b] = branch[c*BC + nb*BSUB + b, (ob*P + p)*BF + f]
        brtc = np.ascontiguousarray(
            br[s].astype(np.float16)
            .reshape(NB, BSUB, OB, P, BF).transpose(0, 2, 3, 4, 1))
        in_maps.append({"wt": wt, "cst": cst,
                        "xt": xtc, "iht": ihtc, "brt": brtc})

    res = run_bass_kernel_spmd(nc, in_maps, list(range(NCORES)), trace=TRACE)
    _CACHE["last"] = res

    out = np.empty((B, OUT), dtype=np.float32)
    for c in range(NCORES):
        # outt[ob, p, nb, b] -> out[c*BC + nb*BSUB + b, ob*P + p]
        ot = res.results[c]["outt"]
        out[c * BC:(c + 1) * BC] = (
            ot.transpose(2, 3, 0, 1).reshape(BC, OUT).astype(np.float32))
    return out


# revision 12
# speedup vs baseline: 1.0184x; 1.0184x over previous
"""Trainium2 Bass kernel for nn_DendriticBranchLayer.

rate = alpha * relu(V - Vth)^2,  V = (exc + cur) / (exc + 1 + cond + inh)
  exc = x @ pruned(pre_w_exc, K=32).T        [B, OUT]
  inh = inhibitory_input @ pruned(pre_w_inh, K=16).T
  cur = sum_f branch_input.reshape(B,OUT,4)[...,f] * w_block[:,f]

Strategy: batch sharded over 8 cores. The masked weights depend only on the
small weight tensors, so dense masked weights are materialized on the host.
Both matmuls run in fp8 E4M3 with perf_mode=DoubleRow (two contraction rows
per PE cell -> 2x matmul throughput vs fp16; measured 512-col DoubleRow MMs
stream at ~216ns = 1 col/cycle with 256-deep contraction). The weights all
lie in exp([-2.1, -2.0]) — a 10%-wide band — so they are rescaled by s to sit
in [0.895, 0.99] where the E4M3 grid is finest, and x is pre-divided by s on
the host (exact scale cancellation). Emulated end-to-end rel_l2: ~1.2e-2
(gate 2e-2). branch_input stays fp16 (it dominates the numerator).

Outputs live on PSUM partitions (128 outputs/block) with 512 batch on the
free dim: one PSUM bank per accumulator, 4+4 banks of double buffering.
Pointwise per (nb, ob) is spread across engines so it hides under the tensor
engine (~83us of matmul stream):
  GpSimd: t0 = br0+br1 ; t1 = br2+br3          (fp16, SBUF-only engine)
  DVE:    cur = t0+t1 ; num = exc_ps + cur ; den = (exc_ps + c) + inh1 ;
          rden = 1/den ; v = num * rden
  ACT:    inh1 = copy(inh_ps) ; r = relu(v - Vth) ; out = (sa*r)^2 -> fp16

DMA is issued from both HWDGE rings (sync + scalar engines) so descriptor
generation is not serialized on one queue; weights are shipped as one merged
768KB tensor per output block. The last iteration runs as two half-batch
pieces so its pointwise tail is half as long.
"""

import numpy as np

import concourse.bass as bass
import concourse.mybir as mybir
import concourse.tile as tile
from concourse import bacc
from concourse.bass_utils import run_bass_kernel_spmd

B, OUT, EXC_IN, INH_IN, BF = 8192, 1024, 4096, 2048, 4
K_EXC, K_INH = 32, 16

NCORES = 8
BC = B // NCORES          # batch per core (1024)
P = 128                   # partitions
NB = 2                    # batch sub-blocks per core
BSUB = BC // NB           # 512 batch per sub-block (one PSUM bank)
OB = OUT // P             # 8 output blocks
KE = EXC_IN // P          # 32 contraction chunks (exc)
KI = INH_IN // P          # 16 contraction chunks (inh)
KW = KE + KI              # merged weight chunks per output block
XCH = 4                   # DMA chunks per x sub-block (8 k-chunks each)
ICH = 4                   # DMA chunks per ih sub-block (4 k-chunks each)

# weight rescale: exp([-2.1,-2.0]) * WS lands in [0.895, 0.989] where the
# E4M3 grid step is 1/16
WS = float(0.9375 / np.exp(-2.05))

# cst column layout: [P, 3*OB]
_C_CP1 = 0                # 1 + cond, per output
_C_VTHN = OB              # -Vth, per output
_C_SA = 2 * OB            # sqrt(alpha), per output
_C_COLS = 3 * OB

_CACHE = {}
TRACE = False  # set by test harness to capture an NTFF profile


def _build_program():
    nc = bacc.Bacc("TRN2", target_bir_lowering=False, debug=False)
    f8, f16, f32 = mybir.dt.float8e4, mybir.dt.float16, mybir.dt.float32
    DR = mybir.MatmulPerfMode.DoubleRow

    wt = nc.declare_dram_parameter("wt", [P, OB, KW, P], f8, isOutput=False)
    xt = nc.declare_dram_parameter("xt", [NB, XCH, P, KE // XCH, BSUB], f8,
                                   isOutput=False)
    iht = nc.declare_dram_parameter("iht", [NB, ICH, P, KI // ICH, BSUB], f8,
                                    isOutput=False)
    brt = nc.declare_dram_parameter("brt", [NB, OB, P, BF, BSUB], f16,
                                    isOutput=False)
    cst = nc.declare_dram_parameter("cst", [P, _C_COLS], f32, isOutput=False)
    outt = nc.declare_dram_parameter("outt", [OB, P, NB, BSUB], f16,
                                     isOutput=True)

    add = mybir.AluOpType.add
    Relu = mybir.ActivationFunctionType.Relu
    Square = mybir.ActivationFunctionType.Square
    Identity = mybir.ActivationFunctionType.Identity

    KEC = KE // XCH           # k-chunks per x DMA chunk (8)
    KIC = KI // ICH           # k-chunks per ih DMA chunk (4)

    with tile.TileContext(nc) as tc:
        with tc.tile_pool(name="wpool", bufs=1) as wpool, \
             tc.tile_pool(name="xpool", bufs=1) as xpool, \
             tc.tile_pool(name="brpool", bufs=5) as brpool, \
             tc.tile_pool(name="wk", bufs=2) as wk, \
             tc.tile_pool(name="opool", bufs=3) as opool, \
             tc.tile_pool(name="ps_exc", bufs=3, space="PSUM") as ps_exc, \
             tc.tile_pool(name="ps_inh", bufs=3, space="PSUM") as ps_inh:

            cst_s = wpool.tile([P, _C_COLS], f32)
            wt_sb = [None] * OB
            xt_sb = [[None] * XCH for _ in range(NB)]
            iht_sb = [[None] * ICH for _ in range(NB)]

            def load_w(ob):
                if ob >= OB or wt_sb[ob] is not None:
                    return
                w = wpool.tile([P, KW, P], f8, tag=f"wt{ob}")
                nc.sync.dma_start(out=w, in_=wt[:, ob, :, :])
                wt_sb[ob] = w

            def load_x(nb, c, eng):
                if xt_sb[nb][c] is not None:
                    return
                xs = xpool.tile([P, KEC, BSUB], f8, tag=f"xt{nb}c{c}")
                eng.dma_start(out=xs, in_=xt[nb, c, :, :, :])
                xt_sb[nb][c] = xs

            def load_ih(nb, c, eng):
                if iht_sb[nb][c] is not None:
                    return
                ihs = xpool.tile([P, KIC, BSUB], f8, tag=f"iht{nb}c{c}")
                eng.dma_start(out=ihs, in_=iht[nb, c, :, :, :])
                iht_sb[nb][c] = ihs

            # critical lead-in, alternating between the two HWDGE rings.
            # ob0 weights as two tiles so the first inh matmuls only gate on
            # the small wti0 slice; sync ring: weights + x, scalar ring:
            # ih + cst (+ br later)
            wi0 = wpool.tile([P, KI, P], f8, tag="wti0")
            nc.sync.dma_start(out=wi0, in_=wt[:, 0, KE:KW, :])
            load_ih(0, 0, nc.scalar)
            we0 = wpool.tile([P, KE, P], f8, tag="wte0")
            nc.sync.dma_start(out=we0, in_=wt[:, 0, 0:KE, :])
            wt_sb[0] = (we0, wi0)
            for c in range(1, ICH):
                load_ih(0, c, nc.scalar)
            for c in range(XCH):
                load_x(0, c, nc.sync)
            nc.scalar.dma_start(out=cst_s, in_=cst[:, :])

            def emit_exc(exc_ps, nb, ob, cs):
                w = wt_sb[ob]
                we = w[0] if isinstance(w, tuple) else w
                for j in range(KE // 2):            # 16 DoubleRow matmuls
                    c, jj = divmod(j, KEC // 2)
                    nc.tensor.matmul(
                        exc_ps,
                        we[:, 2 * j:2 * j + 2, :],
                        xt_sb[nb][c][:, 2 * jj:2 * jj + 2, cs],
                        start=(j == 0), stop=(j == KE // 2 - 1),
                        perf_mode=DR)

            def emit_inh(inh_ps, nb, ob, cs):
                w = wt_sb[ob]
                wi, k0 = (w[1], 0) if isinstance(w, tuple) else (w, KE)
                for j in range(KI // 2):            # 8 DoubleRow matmuls
                    c, jj = divmod(j, KIC // 2)
                    nc.tensor.matmul(
                        inh_ps,
                        wi[:, k0 + 2 * j:k0 + 2 * j + 2, :],
                        iht_sb[nb][c][:, 2 * jj:2 * jj + 2, cs],
                        start=(j == 0), stop=(j == KI // 2 - 1),
                        perf_mode=DR)

            def pointwise(nb, ob, cs, w, sfx, br_s, exc_ps, inh_ps):
                # cur = sum of the 4 branch inputs (w_block folded on host)
                t0 = wk.tile([P, w], f16, tag="t0" + sfx)
                nc.gpsimd.tensor_add(t0, br_s[:, 0, cs], br_s[:, 1, cs])
                t1 = wk.tile([P, w], f16, tag="t1" + sfx)
                nc.gpsimd.tensor_add(t1, br_s[:, 2, cs], br_s[:, 3, cs])
                cur = wk.tile([P, w], f16, tag="cur" + sfx)
                nc.vector.tensor_add(cur, t0, t1)

                num = wk.tile([P, w], f32, tag="num" + sfx)
                nc.vector.tensor_add(num, exc_ps, cur)
                inh1 = wk.tile([P, w], f32, tag="inh1" + sfx)
                nc.scalar.activation(inh1, inh_ps, Identity)
                den = wk.tile([P, w], f32, tag="den" + sfx)
                nc.vector.scalar_tensor_tensor(
                    den, exc_ps, cst_s[:, _C_CP1 + ob:_C_CP1 + ob + 1], inh1,
                    op0=add, op1=add)
                rden = wk.tile([P, w], f32, tag="rden" + sfx)
                nc.vector.reciprocal_approx_fast(rden, den)
                v = wk.tile([P, w], f32, tag="v" + sfx)
                nc.vector.tensor_mul(v, num, rden)
                r = wk.tile([P, w], f32, tag="r" + sfx)
                nc.scalar.activation(
                    r, v, Relu, bias=cst_s[:, _C_VTHN + ob:_C_VTHN + ob + 1])
                ot = opool.tile([P, w], f16, tag="ot" + sfx)
                nc.scalar.activation(
                    ot, r, Square, scale=cst_s[:, _C_SA + ob:_C_SA + ob + 1])
                nc.sync.dma_start(out=outt[ob, :, nb, cs], in_=ot)

            iters = [(n, o) for n in range(NB) for o in range(OB)]
            for it, (nb, ob) in enumerate(iters):
                last = it == len(iters) - 1
                br_s = brpool.tile([P, BF, BSUB], f16, tag="br")
                nc.scalar.dma_start(out=br_s, in_=brt[nb, ob, :, :, :])

                # prefetch: weights early (scalar ring), second batch half
                # by mid-sweep
                if it == 0:
                    load_w(1)
                elif it == 1:
                    load_w(2), load_w(3)
                elif it == 2:
                    for o in range(4, OB):
                        load_w(o)
                elif it in (3, 4, 5, 6):
                    load_x(1, it - 3, nc.sync)
                    load_ih(1, it - 3, nc.sync)

                exc_ps = ps_exc.tile([P, BSUB], f32, tag="exc")
                inh_ps = ps_inh.tile([P, BSUB], f32, tag="inh")
                full = slice(0, BSUB)
                # inh first: the pointwise chain gates on the exc group, so
                # exc must be the last-emitted matmul group of the iteration
                emit_inh(inh_ps, nb, ob, full)
                emit_exc(exc_ps, nb, ob, full)
                if last:
                    # split the final chain so the kernel tail is shorter
                    h = BSUB // 2
                    pointwise(nb, ob, slice(0, h), h, "a", br_s,
                              exc_ps[:, 0:h], inh_ps[:, 0:h])
                    pointwise(nb, ob, slice(h, BSUB), h, "b", br_s,
                              exc_ps[:, h:BSUB], inh_ps[:, h:BSUB])
                else:
                    pointwise(nb, ob, full, BSUB, "", br_s, exc_ps, inh_ps)

    nc.compile()
    return nc


def _pruned_dense_T(pre_w, K):
    """Masked weight, transposed to [in, out] fp32. Tie-break matches
    jax.lax.top_k: equal values -> lower index wins (stable sort)."""
    idx = np.argsort(-pre_w, axis=1, kind="stable")[:, :K]
    w = np.exp(pre_w.astype(np.float32))
    dense = np.zeros(pre_w.shape, dtype=np.float32)
    np.put_along_axis(dense, idx, np.take_along_axis(w, idx, axis=1), axis=1)
    return dense.T


def kernel(x, inhibitory_input, branch_input, pre_w_exc, pre_w_inh,
           w_block, presigmoid_Vth, log_alpha_max):
    if "nc" not in _CACHE:
        _CACHE["nc"] = _build_program()
    nc = _CACHE["nc"]
    f8np = mybir.dt.np(mybir.dt.float8e4)

    x = np.ascontiguousarray(np.asarray(x, dtype=np.float32))
    inh = np.ascontiguousarray(np.asarray(inhibitory_input, dtype=np.float32))
    br = np.ascontiguousarray(np.asarray(branch_input, dtype=np.float32))
    pre_w_exc = np.asarray(pre_w_exc, dtype=np.float32)
    pre_w_inh = np.asarray(pre_w_inh, dtype=np.float32)
    w_block = np.asarray(w_block, dtype=np.float32)
    presigmoid_Vth = np.asarray(presigmoid_Vth, dtype=np.float32)
    log_alpha_max = np.asarray(log_alpha_max, dtype=np.float32)

    # --- replicated operands -------------------------------------------------
    # wt[p, ob, k, o] = WS * W[ob*P + o, k*P + p], fp8; exc chunks 0..KE-1,
    # inh chunks KE..KW-1
    we_t = (_pruned_dense_T(pre_w_exc, K_EXC) * WS).astype(f8np)
    wi_t = (_pruned_dense_T(pre_w_inh, K_INH) * WS).astype(f8np)
    wte = we_t.reshape(KE, P, OB, P).transpose(1, 2, 0, 3)
    wti = wi_t.reshape(KI, P, OB, P).transpose(1, 2, 0, 3)
    wt = np.ascontiguousarray(np.concatenate([wte, wti], axis=2))

    cond = w_block.sum(axis=1, dtype=np.float32)              # [OUT]
    vth = (1.0 / (1.0 + np.exp(-presigmoid_Vth.astype(np.float64)))).astype(np.float32)
    sa = np.sqrt(np.exp(log_alpha_max.astype(np.float32)))
    cst = np.zeros((P, _C_COLS), dtype=np.float32)
    cst[:, _C_CP1:_C_CP1 + OB] = (1.0 + cond).reshape(OB, P).T
    cst[:, _C_VTHN:_C_VTHN + OB] = (-vth).reshape(OB, P).T
    cst[:, _C_SA:_C_SA + OB] = sa.reshape(OB, P).T

    # fold w_block into branch_input (it is all-ones in practice: skip)
    if not np.all(w_block == 1.0):
        br = (br.reshape(B, OUT, BF) * w_block[None]).reshape(B, OUT * BF)

    inv_s = np.float32(1.0 / WS)
    # --- per-core shards -----------------------------------------------------
    in_maps = []
    for c in range(NCORES):
        s = slice(c * BC, (c + 1) * BC)
        # xt[nb, ch, p, k, b] = x[c*BC + nb*BSUB + b, ((ch*KEC)+k)*P + p] / WS
        xtc = np.ascontiguousarray(
            (x[s] * inv_s).astype(f8np)
            .reshape(NB, BSUB, XCH, KE // XCH, P).transpose(0, 2, 4, 3, 1))
        ihtc = np.ascontiguousarray(
            (inh[s] * inv_s).astype(f8np)
            .reshape(NB, BSUB, ICH, KI // ICH, P).transpose(0, 2, 4, 3, 1))
        # brt[nb, ob, p, f, b] = branch[c*BC + nb*BSUB + b, (ob*P + p)*BF + f]
        brtc = np.ascontiguousarray(
            br[s].astype(np.float16)
            .reshape(NB, BSUB, OB, P, BF).transpose(0, 2, 3, 4, 1))
        in_maps.append({"wt": wt, "cst": cst,
                        "xt": xtc, "iht": ihtc, "brt": brtc})

    res = run_bass_kernel_spmd(nc, in_maps, list(range(NCORES)), trace=TRACE)
    _CACHE["last"] = res

    out = np.empty((B, OUT), dtype=np.float32)
    for c in range(NCORES):
        # outt[ob, p, nb, b] -> out[c*BC + nb*BSUB + b, ob*P + p]
        ot = res.results[c]["outt"]
        out[c * BC:(c + 1) * BC] = (
            ot.transpose(2, 3, 0, 1).reshape(BC, OUT).astype(np.float32))
    return out


# revision 13
# speedup vs baseline: 1.0336x; 1.0150x over previous
"""Trainium2 Bass kernel for nn_DendriticBranchLayer.

rate = alpha * relu(V - Vth)^2,  V = (exc + cur) / (exc + 1 + cond + inh)
  exc = x @ pruned(pre_w_exc, K=32).T        [B, OUT]
  inh = inhibitory_input @ pruned(pre_w_inh, K=16).T
  cur = sum_f branch_input.reshape(B,OUT,4)[...,f] * w_block[:,f]

Strategy: batch sharded over 8 cores. The masked weights depend only on the
small weight tensors, so dense masked weights are materialized on the host.
Both matmuls run in fp8 E4M3 with perf_mode=DoubleRow (two contraction rows
per PE cell -> 2x matmul throughput vs fp16; measured 512-col DoubleRow MMs
stream at ~216ns = 1 col/cycle with 256-deep contraction). The weights all
lie in exp([-2.1, -2.0]) — a 10%-wide band — so they are rescaled by s to sit
in [0.895, 0.99] where the E4M3 grid is finest, and x is pre-divided by s on
the host (exact scale cancellation). Emulated end-to-end rel_l2: ~1.2e-2
(gate 2e-2). branch_input stays fp16 (it dominates the numerator).

Outputs live on PSUM partitions (128 outputs/block) with 512 batch on the
free dim: one PSUM bank per accumulator, 4+4 banks of double buffering.
Pointwise per (nb, ob) is spread across engines so it hides under the tensor
engine (~83us of matmul stream):
  GpSimd: t0 = br0+br1 ; t1 = br2+br3          (fp16, SBUF-only engine)
  DVE:    cur = t0+t1 ; num = exc_ps + cur ; den = (exc_ps + c) + inh1 ;
          rden = 1/den ; v = num * rden
  ACT:    inh1 = copy(inh_ps) ; r = relu(v - Vth) ; out = (sa*r)^2 -> fp16

DMA is issued from both HWDGE rings (sync + scalar engines) so descriptor
generation is not serialized on one queue; weights are shipped as one merged
768KB tensor per output block. The last iteration runs as two half-batch
pieces so its pointwise tail is half as long.
"""

import numpy as np

import concourse.bass as bass
import concourse.mybir as mybir
import concourse.tile as tile
from concourse import bacc
from concourse.bass_utils import run_bass_kernel_spmd

B, OUT, EXC_IN, INH_IN, BF = 8192, 1024, 4096, 2048, 4
K_EXC, K_INH = 32, 16

NCORES = 8
BC = B // NCORES          # batch per core (1024)
P = 128                   # partitions
NB = 2                    # batch sub-blocks per core
BSUB = BC // NB           # 512 batch per sub-block (one PSUM bank)
OB = OUT // P             # 8 output blocks
KE = EXC_IN // P          # 32 contraction chunks (exc)
KI = INH_IN // P          # 16 contraction chunks (inh)
KW = KE + KI              # merged weight chunks per output block
XCH = 4                   # DMA chunks per x sub-block (8 k-chunks each)
ICH = 4                   # DMA chunks per ih sub-block (4 k-chunks each)

# weight rescale: exp([-2.1,-2.0]) * WS lands in [0.895, 0.989] where the
# E4M3 grid step is 1/16
WS = float(0.9375 / np.exp(-2.05))

# cst column layout: [P, 3*OB]
_C_CP1 = 0                # 1 + cond, per output
_C_VTHN = OB              # -Vth, per output
_C_SA = 2 * OB            # sqrt(alpha), per output
_C_COLS = 3 * OB

_CACHE = {}
TRACE = False  # set by test harness to capture an NTFF profile


def _build_program():
    nc = bacc.Bacc("TRN2", target_bir_lowering=False, debug=False)
    f8, f16, f32 = mybir.dt.float8e4, mybir.dt.float16, mybir.dt.float32
    DR = mybir.MatmulPerfMode.DoubleRow

    wt = nc.declare_dram_parameter("wt", [P, OB, KW, P], f8, isOutput=False)
    xt = nc.declare_dram_parameter("xt", [NB, XCH, P, KE // XCH, BSUB], f8,
                                   isOutput=False)
    iht = nc.declare_dram_parameter("iht", [NB, ICH, P, KI // ICH, BSUB], f8,
                                    isOutput=False)
    brt = nc.declare_dram_parameter("brt", [NB, OB, P, BF, BSUB], f16,
                                    isOutput=False)
    cst = nc.declare_dram_parameter("cst", [P, _C_COLS], f32, isOutput=False)
    outt = nc.declare_dram_parameter("outt", [OB, P, NB, BSUB], f16,
                                     isOutput=True)

    add = mybir.AluOpType.add
    Relu = mybir.ActivationFunctionType.Relu
    Square = mybir.ActivationFunctionType.Square
    Identity = mybir.ActivationFunctionType.Identity

    KEC = KE // XCH           # k-chunks per x DMA chunk (8)
    KIC = KI // ICH           # k-chunks per ih DMA chunk (4)

    with tile.TileContext(nc) as tc:
        with tc.tile_pool(name="wpool", bufs=1) as wpool, \
             tc.tile_pool(name="xpool", bufs=1) as xpool, \
             tc.tile_pool(name="brpool", bufs=5) as brpool, \
             tc.tile_pool(name="wk", bufs=2) as wk, \
             tc.tile_pool(name="opool", bufs=3) as opool, \
             tc.tile_pool(name="ps_exc", bufs=3, space="PSUM") as ps_exc, \
             tc.tile_pool(name="ps_inh", bufs=3, space="PSUM") as ps_inh:

            cst_s = wpool.tile([P, _C_COLS], f32)
            wt_sb = [None] * OB
            xt_sb = [[None] * XCH for _ in range(NB)]
            iht_sb = [[None] * ICH for _ in range(NB)]

            def load_w(ob):
                if ob >= OB or wt_sb[ob] is not None:
                    return
                w = wpool.tile([P, KW, P], f8, tag=f"wt{ob}")
                nc.sync.dma_start(out=w, in_=wt[:, ob, :, :])
                wt_sb[ob] = w

            def load_x(nb, c, eng):
                if xt_sb[nb][c] is not None:
                    return
                xs = xpool.tile([P, KEC, BSUB], f8, tag=f"xt{nb}c{c}")
                eng.dma_start(out=xs, in_=xt[nb, c, :, :, :])
                xt_sb[nb][c] = xs

            def load_ih(nb, c, eng):
                if iht_sb[nb][c] is not None:
                    return
                ihs = xpool.tile([P, KIC, BSUB], f8, tag=f"iht{nb}c{c}")
                eng.dma_start(out=ihs, in_=iht[nb, c, :, :, :])
                iht_sb[nb][c] = ihs

            # critical lead-in, alternating between the two HWDGE rings.
            # ob0 weights as two tiles so the first inh matmuls only gate on
            # the small wti0 slice; sync ring: weights + x, scalar ring:
            # ih + cst (+ br later)
            wi0 = wpool.tile([P, KI, P], f8, tag="wti0")
            nc.sync.dma_start(out=wi0, in_=wt[:, 0, KE:KW, :])
            load_ih(0, 0, nc.scalar)
            we0 = wpool.tile([P, KE, P], f8, tag="wte0")
            nc.sync.dma_start(out=we0, in_=wt[:, 0, 0:KE, :])
            wt_sb[0] = (we0, wi0)
            for c in range(1, ICH):
                load_ih(0, c, nc.scalar)
            for c in range(XCH):
                load_x(0, c, nc.sync)
            nc.scalar.dma_start(out=cst_s, in_=cst[:, :])

            def emit_exc(exc_ps, nb, ob, cs):
                w = wt_sb[ob]
                we = w[0] if isinstance(w, tuple) else w
                for j in range(KE // 2):            # 16 DoubleRow matmuls
                    c, jj = divmod(j, KEC // 2)
                    nc.tensor.matmul(
                        exc_ps,
                        we[:, 2 * j:2 * j + 2, :],
                        xt_sb[nb][c][:, 2 * jj:2 * jj + 2, cs],
                        start=(j == 0), stop=(j == KE // 2 - 1),
                        perf_mode=DR)

            def emit_inh(inh_ps, nb, ob, cs):
                w = wt_sb[ob]
                wi, k0 = (w[1], 0) if isinstance(w, tuple) else (w, KE)
                for j in range(KI // 2):            # 8 DoubleRow matmuls
                    c, jj = divmod(j, KIC // 2)
                    nc.tensor.matmul(
                        inh_ps,
                        wi[:, k0 + 2 * j:k0 + 2 * j + 2, :],
                        iht_sb[nb][c][:, 2 * jj:2 * jj + 2, cs],
                        start=(j == 0), stop=(j == KI // 2 - 1),
                        perf_mode=DR)

            def pointwise(nb, ob, cs, w, sfx, br_s, exc_ps, inh_ps):
                # cur = sum of the 4 branch inputs (w_block folded on host)
                t0 = wk.tile([P, w], f16, tag="t0" + sfx)
                nc.gpsimd.tensor_add(t0, br_s[:, 0, cs], br_s[:, 1, cs])
                t1 = wk.tile([P, w], f16, tag="t1" + sfx)
                nc.gpsimd.tensor_add(t1, br_s[:, 2, cs], br_s[:, 3, cs])
                cur = wk.tile([P, w], f16, tag="cur" + sfx)
                nc.vector.tensor_add(cur, t0, t1)

                num = wk.tile([P, w], f32, tag="num" + sfx)
                nc.vector.tensor_add(num, exc_ps, cur)
                inh1 = wk.tile([P, w], f32, tag="inh1" + sfx)
                nc.scalar.activation(inh1, inh_ps, Identity)
                den = wk.tile([P, w], f32, tag="den" + sfx)
                nc.vector.scalar_tensor_tensor(
                    den, exc_ps, cst_s[:, _C_CP1 + ob:_C_CP1 + ob + 1], inh1,
                    op0=add, op1=add)
                rden = wk.tile([P, w], f32, tag="rden" + sfx)
                nc.vector.reciprocal_approx_fast(rden, den)
                v = wk.tile([P, w], f32, tag="v" + sfx)
                nc.vector.tensor_mul(v, num, rden)
                r = wk.tile([P, w], f32, tag="r" + sfx)
                nc.scalar.activation(
                    r, v, Relu, bias=cst_s[:, _C_VTHN + ob:_C_VTHN + ob + 1])
                ot = opool.tile([P, w], f16, tag="ot" + sfx)
                nc.scalar.activation(
                    ot, r, Square, scale=cst_s[:, _C_SA + ob:_C_SA + ob + 1])
                nc.sync.dma_start(out=outt[ob, :, nb, cs], in_=ot)

            iters = [(n, o) for n in range(NB) for o in range(OB)]
            full = slice(0, BSUB)
            # inh runs one iteration ahead of exc: each pointwise chain then
            # gates on its exc group only (the conservative matmul-counter
            # watermark keys on emission position), and the tensor stream
            # ends on the last exc group with no trailing inh.
            inh_ps = ps_inh.tile([P, BSUB], f32, tag="inh")
            emit_inh(inh_ps, iters[0][0], iters[0][1], full)
            for it, (nb, ob) in enumerate(iters):
                last = it == len(iters) - 1
                br_s = brpool.tile([P, BF, BSUB], f16, tag="br")
                nc.scalar.dma_start(out=br_s, in_=brt[nb, ob, :, :, :])

                # prefetch: weights early, second batch half by mid-sweep
                if it == 0:
                    load_w(1)
                elif it == 1:
                    load_w(2), load_w(3)
                elif it == 2:
                    for o in range(4, OB):
                        load_w(o)
                elif it in (3, 4, 5, 6):
                    load_x(1, it - 3, nc.sync)
                    load_ih(1, it - 3, nc.sync)

                exc_ps = ps_exc.tile([P, BSUB], f32, tag="exc")
                emit_exc(exc_ps, nb, ob, full)
                if last:
                    # split the final chain so the kernel tail is shorter
                    h = BSUB // 2
                    pointwise(nb, ob, slice(0, h), h, "a", br_s,
                              exc_ps[:, 0:h], inh_ps[:, 0:h])
                    pointwise(nb, ob, slice(h, BSUB), h, "b", br_s,
                              exc_ps[:, h:BSUB], inh_ps[:, h:BSUB])
                else:
                    pointwise(nb, ob, full, BSUB, "", br_s, exc_ps, inh_ps)
                    nxt = ps_inh.tile([P, BSUB], f32, tag="inh")
                    emit_inh(nxt, iters[it + 1][0], iters[it + 1][1], full)
                    inh_ps = nxt

    nc.compile()
    return nc


def _pruned_dense_T(pre_w, K):
    """Masked weight, transposed to [in, out] fp32. Tie-break matches
    jax.lax.top_k: equal values -> lower index wins (stable sort)."""
    idx = np.argsort(-pre_w, axis=1, kind="stable")[:, :K]
    w = np.exp(pre_w.astype(np.float32))
    dense = np.zeros(pre_w.shape, dtype=np.float32)
    np.put_along_axis(dense, idx, np.take_along_axis(w, idx, axis=1), axis=1)
    return dense.T


def kernel(x, inhibitory_input, branch_input, pre_w_exc, pre_w_inh,
           w_block, presigmoid_Vth, log_alpha_max):
    if "nc" not in _CACHE:
        _CACHE["nc"] = _build_program()
    nc = _CACHE["nc"]
    f8np = mybir.dt.np(mybir.dt.float8e4)

    x = np.ascontiguousarray(np.asarray(x, dtype=np.float32))
    inh = np.ascontiguousarray(np.asarray(inhibitory_input, dtype=np.float32))
    br = np.ascontiguousarray(np.asarray(branch_input, dtype=np.float32))
    pre_w_exc = np.asarray(pre_w_exc, dtype=np.float32)
    pre_w_inh = np.asarray(pre_w_inh, dtype=np.float32)
    w_block = np.asarray(w_block, dtype=np.float32)
    presigmoid_Vth = np.asarray(presigmoid_Vth, dtype=np.float32)
    log_alpha_max = np.asarray(log_alpha_max, dtype=np.float32)

    # --- replicated operands -------------------------------------------------
    # wt[p, ob, k, o] = WS * W[ob*P + o, k*P + p], fp8; exc chunks 0..KE-1,
    # inh chunks KE..KW-1
    we_t = (_pruned_dense_T(pre_w_exc, K_EXC) * WS).astype(f8np)
    wi_t = (_pruned_dense_T(pre_w_inh, K_INH) * WS).astype(f8np)
    wte = we_t.reshape(KE, P, OB, P).transpose(1, 2, 0, 3)
    wti = wi_t.reshape(KI, P, OB, P).transpose(1, 2, 0, 3)
    wt = np.ascontiguousarray(np.concatenate([wte, wti], axis=2))

    cond = w_block.sum(axis=1, dtype=np.float32)              # [OUT]
    vth = (1.0 / (1.0 + np.exp(-presigmoid_Vth.astype(np.float64)))).astype(np.float32)
    sa = np.sqrt(np.exp(log_alpha_max.astype(np.float32)))
    cst = np.zeros((P, _C_COLS), dtype=np.float32)
    cst[:, _C_CP1:_C_CP1 + OB] = (1.0 + cond).reshape(OB, P).T
    cst[:, _C_VTHN:_C_VTHN + OB] = (-vth).reshape(OB, P).T
    cst[:, _C_SA:_C_SA + OB] = sa.reshape(OB, P).T

    # fold w_block into branch_input (it is all-ones in practice: skip)
    if not np.all(w_block == 1.0):
        br = (br.reshape(B, OUT, BF) * w_block[None]).reshape(B, OUT * BF)

    inv_s = np.float32(1.0 / WS)
    # --- per-core shards -----------------------------------------------------
    in_maps = []
    for c in range(NCORES):
        s = slice(c * BC, (c + 1) * BC)
        # xt[nb, ch, p, k, b] = x[c*BC + nb*BSUB + b, ((ch*KEC)+k)*P + p] / WS
        xtc = np.ascontiguousarray(
            (x[s] * inv_s).astype(f8np)
            .reshape(NB, BSUB, XCH, KE // XCH, P).transpose(0, 2, 4, 3, 1))
        ihtc = np.ascontiguousarray(
            (inh[s] * inv_s).astype(f8np)
            .reshape(NB, BSUB, ICH, KI // ICH, P).transpose(0, 2, 4, 3, 1))
        # brt[nb, ob, p, f, b] = branch[c*BC + nb*BSUB + b, (ob*P + p)*BF + f]
        brtc = np.ascontiguousarray(
            br[s].astype(np.float16)
            .reshape(NB, BSUB, OB, P, BF).transpose(0, 2, 3, 4, 1))
        in_maps.append({"wt": wt, "cst": cst,
                        "xt": xtc, "iht": ihtc, "brt": brtc})

    res = run_bass_kernel_spmd(nc, in_maps, list(range(NCORES)), trace=TRACE)
    _CACHE["last"] = res

    out = np.empty((B, OUT), dtype=np.float32)
    for c in range(NCORES):
        # outt[ob, p, nb, b] -> out[c*BC + nb*BSUB + b, ob*P + p]
        ot = res.results[c]["outt"]
        out[c * BC:(c + 1) * BC] = (
            ot.transpose(2, 3, 0, 1).reshape(BC, OUT).astype(np.float32))
    return out
